# revision 15
# baseline (speedup 1.0000x reference)
"""Multi-head self-attention (B=4, T=2048, C=1024, H=16, D=64) on 8 NeuronCores.

Sharding: tensor-parallel over heads (Megatron): each core owns 2 heads.
Wq/Wk/Wv column-sharded, Wo row-sharded; host sums the 8 partial outputs.

Device layout is fully "transposed" (features on partitions, tokens on the
free dim) so that softmax runs over the PSUM free dim and the PV matmul needs
no attention-matrix transpose.

Softmax exp is computed three ways, split across engines to break the
Scalar-engine bottleneck:
  - ACT: exact exp -> fp8e4 (activation with scale=1/A, bias shift)
  - DVE/Pool: Schraudolph fast-exp -> int8 bitcast fp8e4:
      bits = clamp(round(A*score + B), 0, 127);  A = 8*log2(e) folded into
      Wq on the host, so the op is a single tensor_scalar add+max with int8
      output.  The int8 bit pattern IS the fp8e4m3 encoding of
      ~exp(score)*2^((B-56)/8), on the same quantization grid the ACT path
      produces; the only extra error is the secant interpolation (~2% rms).
The per-query denominator comes from a stationary 'ones' column in the PV
matmul, so ACT/DVE/Pool tiles stay mutually consistent.

PV runs fp8e4 DoubleRow (2 key-tiles per matmul).  V is split hi/lo with the
128 stationary columns [v_hi(64,perm) | ones(1) | v_lo(63)] where the host
pre-permutes V/Wo dims per head to [63, 0..62].  This makes the PSUM row map
  rows 0:64  = ctx_hi (perm dims 0..63)
  row  64    = denominator
  rows 65:128= ctx_lo (perm dims 1..63)
so hi+lo correction is ONE aligned scalar_tensor_tensor:
  hs = pv[0:64] + pv[64:128]
Perm-dim 0 (orig d63) instead absorbs the denominator: ctx'_0 = ctx_0 + 1,
a constant that the host folds into the output bias (minus sum of the
corresponding Wo rows).

The PE executes its stream in order, so projection/output-projection matmuls
of adjacent batches are interleaved into the attention j-loop to keep the PE
near 100% busy; Wo PSUM drains run on ACT between exp tiles.
"""

import numpy as np

import concourse.bass as bass
import concourse.tile as tile
from concourse import bacc, mybir
from concourse.bass_utils import run_bass_kernel_spmd

B, T, C, H, D = 4, 2048, 1024, 16, 64
NCORES = 8
HPC = H // NCORES          # heads per core = 2
F = HPC * D                # per-core feature width = 128
TT = B * T                 # total tokens = 8192

FP32 = mybir.dt.float32
FP16 = mybir.dt.float16
I8 = mybir.dt.int8
MM_DT = mybir.dt.float16   # matmul compute dtype
PV_DT = mybir.dt.float8e4  # PV matmul dtype (DoubleRow: 2 key-tiles per MM)
OUT_DT = mybir.dt.float16  # partial-output DMA dtype

TILE_K = 128               # contraction tile
TILE_N = 512               # moving free dim per matmul
NK_C = C // TILE_K         # 8 k-tiles over channels
NT4 = T // TILE_N          # 4 token chunks per batch
NJ = T // TILE_K           # 16 key tiles per batch
NP = NJ // 2               # 8 key-tile PAIRS per batch (DoubleRow)
NLO = D - 1                # 63 lo-correction dims (perm-dim 0 is hi-only)

A_BITS = 8.0 / np.log(2.0)          # 11.5416: fp8e4m3 bits per ln unit
B_BITS = 24.0                       # bias anchor: max P byte 118 < 0x78
# CRITICAL: the PE fp8e4m3 decoder treats exponent-field 1111 (any byte
# >= 0x78, value >= 256) as Inf/NaN -- P values must stay < 256.
# DVE fast-exp bias, recentered by the mean secant error so DVE tiles match
# the ACT-exact tiles' scale inside one softmax row (swept in simulation).
B_DVE = 23.5
EXP_BIAS = (B_BITS - 56.0) / 8.0 * np.log(2.0)  # ACT-exp shift, same scale
EXP_SCALE = 1.0 / A_BITS            # undo the A folded into Wq

# exp-engine rotation per chunk: 16 tiles (8 pair-steps x 2 j2)
# A=ACT exact exp, D=DVE fast-exp (Pool cannot read PSUM)
EXP_PAT = ['A', 'D', 'A', 'D', 'A', 'D', 'A', 'D',
           'A', 'D', 'A', 'D', 'A', 'D', 'A', 'A']
# Wo PSUM drain engine per o-tile (6 ACT / 2 DVE per chunk)
OSB_PAT = ['A'] * 8


def build_kernel_body(tc):
    nc = tc.nc
    Exp = mybir.ActivationFunctionType.Exp
    Copy = mybir.ActivationFunctionType.Copy
    Ident = mybir.ActivationFunctionType.Identity
    DR = mybir.MatmulPerfMode.DoubleRow
    ADD = mybir.AluOpType.add
    MAX = mybir.AluOpType.max
    MULT = mybir.AluOpType.mult

    xT = nc.dram_tensor("xT", [C, TT], MM_DT, kind="ExternalInput").ap()
    wq = nc.dram_tensor("wq", [C, F], MM_DT, kind="ExternalInput").ap()
    wk = nc.dram_tensor("wk", [C, F], MM_DT, kind="ExternalInput").ap()
    wv = nc.dram_tensor("wv", [C, F], MM_DT, kind="ExternalInput").ap()
    wo = nc.dram_tensor("wo", [F, C], MM_DT, kind="ExternalInput").ap()
    bqv = nc.dram_tensor("bq", [F], FP32, kind="ExternalInput").ap()
    bkv = nc.dram_tensor("bk", [F], FP32, kind="ExternalInput").ap()
    outT = nc.dram_tensor("outT", [C, TT], OUT_DT, kind="ExternalOutput").ap()


    import contextlib
    ctx = contextlib.ExitStack()
    with ctx:
        consts = ctx.enter_context(tc.tile_pool(name="consts", bufs=1))
        xpool = ctx.enter_context(tc.tile_pool(name="xt", bufs=64))
        bigs = ctx.enter_context(tc.tile_pool(name="bigs", bufs=2))
        epool = ctx.enter_context(tc.tile_pool(name="expp", bufs=10))
        small = ctx.enter_context(tc.tile_pool(name="small", bufs=2))
        vstage = ctx.enter_context(tc.tile_pool(name="vstage", bufs=2))
        ps_qk = ctx.enter_context(tc.tile_pool(name="ps_qk", bufs=2, space="PSUM"))
        ps_pv = ctx.enter_context(tc.tile_pool(name="ps_pv", bufs=1, space="PSUM"))
        ps_aux = ctx.enter_context(tc.tile_pool(name="ps_aux", bufs=2, space="PSUM"))

        # ---- constants ----
        wq_sb = consts.tile([TILE_K, C], MM_DT)  # c-tile k at [:, k*F:(k+1)*F]
        nc.sync.dma_start(
            wq_sb[:].rearrange("p (k f) -> p k f", k=NK_C),
            wq.rearrange("(k p) f -> p k f", p=TILE_K))
        wk_sb = consts.tile([TILE_K, C], MM_DT)
        nc.sync.dma_start(
            wk_sb[:].rearrange("p (k f) -> p k f", k=NK_C),
            wk.rearrange("(k p) f -> p k f", p=TILE_K))
        wv_sb = consts.tile([TILE_K, C], MM_DT)
        nc.sync.dma_start(
            wv_sb[:].rearrange("p (k f) -> p k f", p=TILE_K, k=NK_C),
            wv.rearrange("(k p) f -> p k f", p=TILE_K))
        wo_sb = consts.tile([F, C], MM_DT)
        nc.sync.dma_start(wo_sb[:], wo)
        bq_sb = consts.tile([F, 1], FP32)
        nc.sync.dma_start(bq_sb[:], bqv.rearrange("(p one) -> p one", one=1))
        bk_sb = consts.tile([F, 1], FP32)
        nc.sync.dma_start(bk_sb[:], bkv.rearrange("(p one) -> p one", one=1))
        ident32 = consts.tile([128, 128], FP32)
        from concourse.masks import make_identity
        make_identity(nc, ident32[:])
        ident = consts.tile([128, 128], MM_DT)
        nc.vector.tensor_copy(ident[:], ident32[:])
        ones8 = consts.tile([128, NJ * HPC], PV_DT)
        nc.gpsimd.memset(ones8[:], 1.0)
        nexp = consts.tile([128, 1], FP32)
        nc.gpsimd.memset(nexp[:], float(EXP_BIAS))

        tiles = {}  # per-batch SBUF tiles

        def alloc_proj_tiles(b):
            qT = bigs.tile([F, T], MM_DT, tag="qT", name=f"qT{b}")
            kT = bigs.tile([F, T], MM_DT, tag="kT", name=f"kT{b}")
            # PV stationary, both heads: [keys, ktile, head, hi|ones|lo]
            v1 = bigs.tile([128, NJ, HPC, 128], PV_DT, tag="v1",
                           name=f"v1_{b}")
            nc.vector.tensor_copy(
                v1[:, :, :, D : D + 1],
                ones8[:].rearrange("p (a h b) -> p a h b", h=HPC, b=1),
            )
            tiles[b] = {"qT": qT, "kT": kT, "v1": v1}

        xstage = {}

        def gen_xt(b):
            """Generator: issue batch b's x DMA loads, spread out so the
            Sync queue is not clogged by a 32-DMA burst."""
            t0 = b * T
            for t4 in range(NT4):
                for kk in range(NK_C):
                    xt = xpool.tile([TILE_K, TILE_N], MM_DT, tag="xt",
                                    name=f"xt{b}_{t4}_{kk}")
                    nc.sync.dma_start(
                        xt[:],
                        xT[kk * TILE_K : (kk + 1) * TILE_K,
                           t0 + t4 * TILE_N : t0 + (t4 + 1) * TILE_N],
                    )
                    xstage[b, t4, kk] = xt
                    if kk % 4 == 3:
                        yield

        def prefetch_x(b):
            for _ in gen_xt(b):
                pass

        xhold = {}

        def gen_proj(b, defer_q=False):
            """Generator: projections for batch b; yields after each PE op."""
            alloc_proj_tiles(b)
            tl = tiles[b]
            for t4 in range(NT4):
                xts = [xstage.pop((b, t4, kk)) for kk in range(NK_C)]
                wlist = (("q", wq_sb), ("k", wk_sb), ("v", wv_sb))
                if defer_q and t4 > 0:
                    xhold[b, t4] = xts
                    wlist = (("k", wk_sb), ("v", wv_sb))
                for which, w_sb in wlist:
                    acc = ps_aux.tile([128, TILE_N], FP32, tag="aux")
                    for kk in range(NK_C):
                        nc.tensor.matmul(
                            acc[:], w_sb[:, kk * F : (kk + 1) * F], xts[kk][:],
                            start=(kk == 0), stop=(kk == NK_C - 1),
                        )
                        yield
                    if which == "q":
                        nc.vector.tensor_scalar_add(
                            tl["qT"][:, t4 * TILE_N : (t4 + 1) * TILE_N],
                            acc[:], bq_sb[:])
                    elif which == "k":
                        nc.vector.tensor_scalar_add(
                            tl["kT"][:, t4 * TILE_N : (t4 + 1) * TILE_N],
                            acc[:], bk_sb[:])
                    else:
                        vt_sb = vstage.tile([128, TILE_N], MM_DT, tag="vt")
                        nc.vector.tensor_copy(vt_sb[:], acc[:])
                        # transpose [dims, tokens] -> [tokens, dims] on the
                        # PE; all four 128-blocks share one PSUM slot, then
                        # one fast high-priority copy releases it
                        ptr4 = ps_aux.tile([128, TILE_N], MM_DT, tag="aux")
                        for tt in range(TILE_N // 128):
                            nc.tensor.transpose(
                                ptr4[:, tt * 128 : (tt + 1) * 128],
                                vt_sb[:, tt * 128 : (tt + 1) * 128], ident[:])
                            yield
                        vUT = vstage.tile([128, TILE_N // 128, 128], MM_DT,
                                          tag="vUT", bufs=2)
                        nc.vector.tensor_copy(
                            vUT[:].rearrange("p b d -> p (b d)"), ptr4[:])
                        # v1 fp8 hi/lo split, whole 512-token chunk at once.
                        # Host pre-permuted V dims to [63, 0..62]; lo covers
                        # perm dims 1..63 and is written as fp8 directly.
                        vh = tl["v1"]
                        jsl4 = slice(t4 * 4, t4 * 4 + 4)
                        v4 = vUT[:].rearrange("p b (h d) -> p b h d", h=HPC)
                        # hi: fp8 round of v (all 64 perm dims)
                        nc.vector.tensor_copy(vh[:, jsl4, :, 0:D], v4)
                        # lo: v - hi (perm dims 1..63), fp8 output directly
                        nc.vector.tensor_sub(
                            vh[:, jsl4, :, D + 1 : 128],
                            v4[:, :, :, 1:D], vh[:, jsl4, :, 1:D])

        def gen_q_rest(b):
            """Generator: deferred q projections for chunks 1..3 of batch b."""
            tl = tiles[b]
            for t4 in range(1, NT4):
                xts = xhold.pop((b, t4))
                acc = ps_aux.tile([128, TILE_N], FP32, tag="aux")
                for kk in range(NK_C):
                    nc.tensor.matmul(
                        acc[:], wq_sb[:, kk * F : (kk + 1) * F], xts[kk][:],
                        start=(kk == 0), stop=(kk == NK_C - 1),
                    )
                    yield
                nc.vector.tensor_scalar_add(
                    tl["qT"][:, t4 * TILE_N : (t4 + 1) * TILE_N],
                    acc[:], bq_sb[:])

        def gen_wo_t4(b, t4):
            """Generator: output projection chunk; yields per PE op.
            PSUM drain runs on ACT (between exp tiles)."""
            t0 = b * T
            ctxT = tiles[b]["ctxT"]
            for o in range(C // 128):
                po = ps_aux.tile([128, TILE_N], FP32, tag="aux")
                nc.tensor.matmul(
                    po[:], wo_sb[:, o * 128 : (o + 1) * 128],
                    ctxT[:, t4 * TILE_N : (t4 + 1) * TILE_N],
                    start=True, stop=True,
                )
                osb = vstage.tile([128, TILE_N], OUT_DT, tag="osb", bufs=4)
                if OSB_PAT[o] == 'A':
                    nc.scalar.activation(osb[:], po[:], Copy)
                else:
                    nc.vector.tensor_copy(osb[:], po[:])
                nc.sync.dma_start(
                    outT[o * 128 : (o + 1) * 128,
                         t0 + t4 * TILE_N : t0 + (t4 + 1) * TILE_N],
                    osb[:],
                )
                yield

        fillers = []

        def pull(budget):
            while budget > 0 and fillers:
                try:
                    next(fillers[0])
                    budget -= 1
                except StopIteration:
                    fillers.pop(0)

        # prologue: projections for batch 0 (PE-only ramp); batch 1's
        # x-loads and projection generator queue up behind it
        prefetch_x(0)
        prefetch_x(1)
        for _ in gen_proj(0, defer_q=True):
            pass
        fillers.append(gen_q_rest(0))
        fillers.append(gen_proj(1))

        # ---- flattened software-pipelined attention stream ----
        # one pair-step = 2 key tiles x 2 heads: 2x(QK pair + exp), then the
        # (a few steps earlier, per pv_sched) DoubleRow PV pair.  Chunks of
        # 512 queries flow back-to-back so no engine stalls at chunk borders.
        chunks = [(b, i4) for b in range(B) for i4 in range(NT4)]
        S = len(chunks) * NP
        pvs = {}     # chunk idx -> (pv0, pv1)
        expts = {}   # (chunk, pair) -> epair tile

        # pv pair -> emission step: finish each chunk's PV one step into the
        # next chunk so the normalize chain gets ~2 steps before the PSUM
        # banks are rewritten; chunk 0 lags more (v1 is still being built).
        from collections import defaultdict
        pv_sched = defaultdict(list)
        for c in range(len(chunks)):
            offs = [4, 5, 6, 7, 8, 9, 9, 10]
            for p, off in enumerate(offs):
                pv_sched[8 * c + off].append((c, p))
        S_END = max(pv_sched) + 1

        def chunk_start(ci):
            b, i4 = chunks[ci]
            if i4 == 0:
                tiles[b]["ctxT"] = bigs.tile([F, T], MM_DT, tag="ctxT",
                                             name=f"ctxT{b}")
            # queue the next batch's x loads (spread across pulls) and
            # projections at the start of this batch's attention;
            # batch 1 was queued by the prologue
            if i4 == 0 and 1 < b + 1 < B:
                fillers.append(gen_xt(b + 1))
                fillers.append(gen_proj(b + 1))

        def pv_step(ci, pp):
            b, i4 = chunks[ci]
            v1 = tiles[b]["v1"]
            if pp == 0:
                pvs[ci] = (ps_pv.tile([128, TILE_N], FP32, tag="pv0",
                                      name=f"pv0_{ci}"),
                           ps_pv.tile([128, TILE_N], FP32, tag="pv1",
                                      name=f"pv1_{ci}"))
            e = expts.pop((ci, pp))
            for h, pv in ((0, pvs[ci][0]), (1, pvs[ci][1])):
                nc.tensor.matmul(
                    pv[:], v1[:, 2 * pp : 2 * pp + 2, h, :],
                    e[:, :, h, :], start=(pp == 0), stop=(pp == NP - 1),
                    perf_mode=DR)
            if pp == NP - 1:
                chunk_finish(ci)

        def chunk_finish(ci):
            b, i4 = chunks[ci]
            isl = slice(i4 * TILE_N, (i4 + 1) * TILE_N)
            ctxT = tiles[b]["ctxT"]
            pv0, pv1 = pvs.pop(ci)
            # psum rows: 0:64 ctx_hi (perm dims), 64 denom, 65:128 ctx_lo
            # (perm dims 1..63).  hi+lo in one aligned stt per head; perm
            # dim 0 absorbs the denominator (+1, host-folded).  Bank is
            # released after its stt + recip reads.
            rd = small.tile([1, HPC, TILE_N], FP32, tag="rd")
            hs = small.tile([D, HPC, TILE_N], FP32, tag="hs")
            lo_s = small.tile([D, HPC, TILE_N], FP32, tag="lo_s")
            for h, pv in ((0, pv0), (1, pv1)):
                # partition-shift copy (quadrant base 64 -> 0), then the
                # hi+lo fold reads PSUM + SBUF at matching base 0
                nc.vector.tensor_copy(lo_s[:, h, :], pv[D:128, :])
                nc.vector.scalar_tensor_tensor(
                    hs[:, h, :], pv[0:D, :], 1.0, lo_s[:, h, :], MULT, ADD)
                # denom is row 0 of the shifted lo_s copy -> recip reads SBUF
                nc.vector.reciprocal_approx_fast(rd[:, h, :], lo_s[0:1, h, :])
            bc = small.tile([D, HPC, TILE_N], FP32, tag="bc")
            nc.gpsimd.partition_broadcast(bc[:], rd[:])
            for h in range(HPC):
                nc.gpsimd.tensor_tensor(
                    ctxT[h * D : (h + 1) * D, isl], hs[:, h, :], bc[:, h, :],
                    MULT)
            if b == B - 1 and fillers:
                fillers.insert(1, gen_wo_t4(b, i4))
            else:
                fillers.append(gen_wo_t4(b, i4))

        for s in range(S_END):
            due = pv_sched.get(s, [])
            if s < S:
                ci, p = divmod(s, NP)
                b, i4 = chunks[ci]
                if p == 0:
                    chunk_start(ci)
                qT, kT = tiles[b]["qT"], tiles[b]["kT"]
                isl = slice(i4 * TILE_N, (i4 + 1) * TILE_N)
                epair = epool.tile([128, 2, HPC, TILE_N], PV_DT, tag="expt")
                expts[ci, p] = epair
                for j2 in range(2):
                    j = 2 * p + j2
                    jsl = slice(j * TILE_K, (j + 1) * TILE_K)
                    qk = ps_qk.tile([128, HPC, TILE_N], FP32, tag="qk")
                    # heads in distinct PE row-groups -> run concurrently
                    nc.tensor.matmul(qk[:, 0, :], kT[0:D, jsl],
                                     qT[0:D, isl], start=True, stop=True)
                    nc.tensor.matmul(qk[:, 1, :],
                                     kT[D : 2 * D, jsl], qT[D : 2 * D, isl],
                                     start=True, stop=True)
                    eng = EXP_PAT[(p * 2 + j2) % 16]
                    eslice = epair[:, j2]
                    if eng == 'A':
                        nc.scalar.activation(eslice, qk[:], Exp,
                                             bias=nexp[:], scale=EXP_SCALE)
                    else:
                        nc.vector.tensor_scalar(
                            eslice.bitcast(I8), qk[:], B_DVE, 0.0, ADD, MAX)
                    # keep the in-order PE stream fed while exp runs
                    if j2 == 0:
                        for c2, p2 in due[0:1]:
                            pv_step(c2, p2)
                    else:
                        for c2, p2 in due[1:]:
                            pv_step(c2, p2)
                    pull(2 if j2 == 0 else 3)
            else:
                for c2, p2 in due:
                    pv_step(c2, p2)
                pull(6)

        # drain remaining fillers (last batch's final wo chunks)
        pull(10 ** 9)


_CACHE = {}


def _get_nc():
    if "nc" not in _CACHE:
        nc = bacc.Bacc("TRN2", target_bir_lowering=False, debug=False,
                       num_devices=NCORES)
        with tile.TileContext(nc) as tc:
            build_kernel_body(tc)
        nc.compile()
        _CACHE["nc"] = nc
    return _CACHE["nc"]


# per-head dim permutation [63, 0..62]: perm-dim 0 absorbs the denominator
_PERM = np.concatenate([[D - 1], np.arange(D - 1)])


def host_prep(x, Wq, bq, Wk, bk, Wv, bv, Wo, bo):
    f16 = np.float16
    x = np.asarray(x, np.float32)
    xT = np.ascontiguousarray(x.reshape(TT, C).T.astype(f16))
    scale = np.float32(A_BITS / np.sqrt(D))
    Wv32 = np.asarray(Wv, np.float32)
    Wo32 = np.asarray(Wo, np.float32)
    # permute V output dims (and matching Wo input dims) per head
    perm_all = np.concatenate([h * D + _PERM for h in range(H)])
    Wv_p = Wv32[:, perm_all]
    Wo_p = Wo32[perm_all, :]
    in_maps = []
    for c in range(NCORES):
        fsl = slice(c * F, (c + 1) * F)
        in_maps.append({
            "xT": xT,
            "wq": np.ascontiguousarray(
                (np.asarray(Wq, np.float32)[:, fsl] * scale).astype(f16)),
            "wk": np.ascontiguousarray(np.asarray(Wk, np.float32)[:, fsl].astype(f16)),
            "wv": np.ascontiguousarray(Wv_p[:, fsl].astype(f16)),
            "wo": np.ascontiguousarray(Wo_p[fsl, :].astype(f16)),
            "bq": np.ascontiguousarray(np.asarray(bq, np.float32)[fsl] * scale),
            "bk": np.ascontiguousarray(np.asarray(bk, np.float32)[fsl]),
        })
    return in_maps


def host_gather(results, Wo, bo, bv):
    total = np.zeros((C, TT), np.float32)
    for c in range(NCORES):
        total += results[c]["outT"].astype(np.float32)
    out = total.T
    Wo32 = np.asarray(Wo, np.float32)
    # +1 artifact on perm-dim 0 (orig d63) of every head
    ones_fix = Wo32[[h * D + (D - 1) for h in range(H)], :].sum(axis=0)
    out = out + (np.asarray(bo, np.float32)
                 + np.asarray(bv, np.float32) @ Wo32 - ones_fix)
    return out.reshape(B, T, C)


def _install_profile_hook():
    """Make trace=True work under axon when antenv.axon_hooks is absent."""
    import sys
    import types

    try:
        import antenv.axon_hooks  # noqa: F401
        return
    except ImportError:
        pass
    import antenv
    from trn_agent_boot.trn_boot import _ntff_profile_via_ctypes

    mod = types.ModuleType("antenv.axon_hooks")
    holder = [None]
    mod.set_axon_ntff_profile_hook = lambda h: holder.__setitem__(0, h)
    mod.get_axon_ntff_profile_hook = lambda: holder[0]
    sys.modules["antenv.axon_hooks"] = mod
    antenv.axon_hooks = mod
    mod.set_axon_ntff_profile_hook(
        _ntff_profile_via_ctypes("/opt/axon/libaxon_pjrt.so")
    )
    # artifact upload needs internal storage; keep profiles local
    import concourse.bass_utils as bu
    bu.upload_artifacts = lambda tmpdir: f"local:{tmpdir}"


def kernel(x, Wq, bq, Wk, bk, Wv, bv, Wo, bo, _trace=False):
    if _trace:
        _install_profile_hook()
    nc = _get_nc()
    in_maps = host_prep(x, Wq, bq, Wk, bk, Wv, bv, Wo, bo)
    res = run_bass_kernel_spmd(nc, in_maps, core_ids=list(range(NCORES)),
                               trace=_trace)
    _CACHE["last_result"] = res
    return host_gather(res.results, Wo, bo, bv)


# revision 17
# speedup vs baseline: 1.1735x; 1.1735x over previous
"""Multi-head self-attention (B=4, T=2048, C=1024, H=16, D=64) on 8 NeuronCores.

Sharding: tensor-parallel over heads (Megatron): each core owns 2 heads.
Wq/Wk/Wv column-sharded, Wo row-sharded; host sums the 8 partial outputs.

Device layout is fully "transposed" (features on partitions, tokens on the
free dim) so that softmax runs over the PSUM free dim and the PV matmul needs
no attention-matrix transpose.

Softmax exp is computed three ways, split across engines to break the
Scalar-engine bottleneck:
  - ACT: exact exp -> fp8e4 (activation with scale=1/A, bias shift)
  - DVE/Pool: Schraudolph fast-exp -> int8 bitcast fp8e4:
      bits = clamp(round(A*score + B), 0, 127);  A = 8*log2(e) folded into
      Wq on the host, so the op is a single tensor_scalar add+max with int8
      output.  The int8 bit pattern IS the fp8e4m3 encoding of
      ~exp(score)*2^((B-56)/8), on the same quantization grid the ACT path
      produces; the only extra error is the secant interpolation (~2% rms).
The per-query denominator comes from a stationary 'ones' column in the PV
matmul, so ACT/DVE/Pool tiles stay mutually consistent.

PV runs fp8e4 DoubleRow (2 key-tiles per matmul).  V is split hi/lo with the
128 stationary columns [v_hi(64,perm) | ones(1) | v_lo(63)] where the host
pre-permutes V/Wo dims per head to [63, 0..62].  This makes the PSUM row map
  rows 0:64  = ctx_hi (perm dims 0..63)
  row  64    = denominator
  rows 65:128= ctx_lo (perm dims 1..63)
so hi+lo correction is ONE aligned scalar_tensor_tensor:
  hs = pv[0:64] + pv[64:128]
Perm-dim 0 (orig d63) instead absorbs the denominator: ctx'_0 = ctx_0 + 1,
a constant that the host folds into the output bias (minus sum of the
corresponding Wo rows).

The PE executes its stream in order, so projection/output-projection matmuls
of adjacent batches are interleaved into the attention j-loop to keep the PE
near 100% busy; Wo PSUM drains run on ACT between exp tiles.
"""

import numpy as np

import concourse.bass as bass
import concourse.tile as tile
from concourse import bacc, mybir
from concourse.bass_utils import run_bass_kernel_spmd

B, T, C, H, D = 4, 2048, 1024, 16, 64
NCORES = 8
HPC = H // NCORES          # heads per core = 2
F = HPC * D                # per-core feature width = 128
TT = B * T                 # total tokens = 8192

FP32 = mybir.dt.float32
FP16 = mybir.dt.float16
I8 = mybir.dt.int8
MM_DT = mybir.dt.float16   # matmul compute dtype
PV_DT = mybir.dt.float8e4  # PV matmul dtype (DoubleRow: 2 key-tiles per MM)
OUT_DT = mybir.dt.float16  # partial-output DMA dtype

TILE_K = 128               # contraction tile
TILE_N = 512               # moving free dim per matmul
NK_C = C // TILE_K         # 8 k-tiles over channels
NT4 = T // TILE_N          # 4 token chunks per batch
NJ = T // TILE_K           # 16 key tiles per batch
NP = NJ // 2               # 8 key-tile PAIRS per batch (DoubleRow)
NLO = D - 1                # 63 lo-correction dims (perm-dim 0 is hi-only)

A_BITS = 8.0 / np.log(2.0)          # 11.5416: fp8e4m3 bits per ln unit
B_BITS = 24.0                       # bias anchor: max P byte 118 < 0x78
# CRITICAL: the PE fp8e4m3 decoder treats exponent-field 1111 (any byte
# >= 0x78, value >= 256) as Inf/NaN -- P values must stay < 256.
# DVE fast-exp bias, recentered by the mean secant error so DVE tiles match
# the ACT-exact tiles' scale inside one softmax row (swept in simulation).
B_DVE = 23.6
EXP_BIAS = (B_BITS - 56.0) / 8.0 * np.log(2.0)  # ACT-exp shift, same scale
EXP_SCALE = 1.0 / A_BITS            # undo the A folded into Wq

# exp-engine rotation per chunk: 16 tiles (8 pair-steps x 2 j2)
# A=ACT exact exp, D=DVE fast-exp (Pool cannot read PSUM)
EXP_PAT = ['A', 'D', 'A', 'A', 'D', 'A', 'A', 'D',
           'A', 'A', 'D', 'A', 'A', 'D', 'A', 'A']
# Wo PSUM drain engine per o-tile (6 ACT / 2 DVE per chunk)
OSB_PAT = ['A', 'A', 'A', 'D', 'A', 'A', 'A', 'D']


def build_kernel_body(tc):
    nc = tc.nc
    Exp = mybir.ActivationFunctionType.Exp
    Copy = mybir.ActivationFunctionType.Copy
    Ident = mybir.ActivationFunctionType.Identity
    DR = mybir.MatmulPerfMode.DoubleRow
    ADD = mybir.AluOpType.add
    MAX = mybir.AluOpType.max
    MULT = mybir.AluOpType.mult

    xT = nc.dram_tensor("xT", [C, TT], MM_DT, kind="ExternalInput").ap()
    wq = nc.dram_tensor("wq", [C, F], MM_DT, kind="ExternalInput").ap()
    wk = nc.dram_tensor("wk", [C, F], MM_DT, kind="ExternalInput").ap()
    wv = nc.dram_tensor("wv", [C, F], MM_DT, kind="ExternalInput").ap()
    wo = nc.dram_tensor("wo", [F, C], MM_DT, kind="ExternalInput").ap()
    bqv = nc.dram_tensor("bq", [F], FP32, kind="ExternalInput").ap()
    bkv = nc.dram_tensor("bk", [F], FP32, kind="ExternalInput").ap()
    outT = nc.dram_tensor("outT", [C, TT], OUT_DT, kind="ExternalOutput").ap()


    import contextlib
    ctx = contextlib.ExitStack()
    with ctx:
        consts = ctx.enter_context(tc.tile_pool(name="consts", bufs=1))
        xpool = ctx.enter_context(tc.tile_pool(name="xt", bufs=64))
        bigs = ctx.enter_context(tc.tile_pool(name="bigs", bufs=2))
        epool = ctx.enter_context(tc.tile_pool(name="expp", bufs=8))
        small = ctx.enter_context(tc.tile_pool(name="small", bufs=2))
        vstage = ctx.enter_context(tc.tile_pool(name="vstage", bufs=2))
        ps_qk = ctx.enter_context(tc.tile_pool(name="ps_qk", bufs=2, space="PSUM"))
        ps_pv = ctx.enter_context(tc.tile_pool(name="ps_pv", bufs=1, space="PSUM"))
        ps_aux = ctx.enter_context(tc.tile_pool(name="ps_aux", bufs=2, space="PSUM"))

        # ---- constants ----
        wq_sb = consts.tile([TILE_K, C], MM_DT)  # c-tile k at [:, k*F:(k+1)*F]
        nc.sync.dma_start(
            wq_sb[:].rearrange("p (k f) -> p k f", k=NK_C),
            wq.rearrange("(k p) f -> p k f", p=TILE_K))
        wk_sb = consts.tile([TILE_K, C], MM_DT)
        nc.sync.dma_start(
            wk_sb[:].rearrange("p (k f) -> p k f", k=NK_C),
            wk.rearrange("(k p) f -> p k f", p=TILE_K))
        wv_sb = consts.tile([TILE_K, C], MM_DT)
        nc.sync.dma_start(
            wv_sb[:].rearrange("p (k f) -> p k f", p=TILE_K, k=NK_C),
            wv.rearrange("(k p) f -> p k f", p=TILE_K))
        wo_sb = consts.tile([F, C], MM_DT)
        nc.sync.dma_start(wo_sb[:], wo)
        bq_sb = consts.tile([F, 1], FP32)
        nc.sync.dma_start(bq_sb[:], bqv.rearrange("(p one) -> p one", one=1))
        bk_sb = consts.tile([F, 1], FP32)
        nc.sync.dma_start(bk_sb[:], bkv.rearrange("(p one) -> p one", one=1))
        ident32 = consts.tile([128, 128], FP32)
        from concourse.masks import make_identity
        make_identity(nc, ident32[:])
        ident = consts.tile([128, 128], MM_DT)
        nc.vector.tensor_copy(ident[:], ident32[:])
        ones8 = consts.tile([128, NJ * HPC], PV_DT)
        nc.gpsimd.memset(ones8[:], 1.0)
        nexp = consts.tile([128, 1], FP32)
        nc.gpsimd.memset(nexp[:], float(EXP_BIAS))

        tiles = {}  # per-batch SBUF tiles

        def alloc_proj_tiles(b):
            qT = bigs.tile([F, T], MM_DT, tag="qT", name=f"qT{b}")
            kT = bigs.tile([F, T], MM_DT, tag="kT", name=f"kT{b}")
            # PV stationary, both heads: [keys, ktile, head, hi|ones|lo]
            v1 = bigs.tile([128, NJ, HPC, 128], PV_DT, tag="v1",
                           name=f"v1_{b}")
            nc.vector.tensor_copy(
                v1[:, :, :, D : D + 1],
                ones8[:].rearrange("p (a h b) -> p a h b", h=HPC, b=1),
            )
            tiles[b] = {"qT": qT, "kT": kT, "v1": v1}

        xstage = {}

        def gen_xt(b):
            """Generator: issue batch b's x DMA loads, spread out so the
            Sync queue is not clogged by a 32-DMA burst."""
            t0 = b * T
            for t4 in range(NT4):
                for kk in range(NK_C):
                    xt = xpool.tile([TILE_K, TILE_N], MM_DT, tag="xt",
                                    name=f"xt{b}_{t4}_{kk}")
                    nc.sync.dma_start(
                        xt[:],
                        xT[kk * TILE_K : (kk + 1) * TILE_K,
                           t0 + t4 * TILE_N : t0 + (t4 + 1) * TILE_N],
                    )
                    xstage[b, t4, kk] = xt
                    if kk % 4 == 3:
                        yield

        def prefetch_x(b):
            for _ in gen_xt(b):
                pass

        xhold = {}

        def gen_proj(b, defer_q=False):
            """Generator: projections for batch b; yields after each PE op."""
            alloc_proj_tiles(b)
            tl = tiles[b]
            for t4 in range(NT4):
                xts = [xstage.pop((b, t4, kk)) for kk in range(NK_C)]
                wlist = (("q", wq_sb), ("k", wk_sb), ("v", wv_sb))
                if defer_q and t4 > 0:
                    xhold[b, t4] = xts
                    wlist = (("k", wk_sb), ("v", wv_sb))
                for which, w_sb in wlist:
                    acc = ps_aux.tile([128, TILE_N], FP32, tag="aux")
                    for kk in range(NK_C):
                        nc.tensor.matmul(
                            acc[:], w_sb[:, kk * F : (kk + 1) * F], xts[kk][:],
                            start=(kk == 0), stop=(kk == NK_C - 1),
                        )
                        yield
                    if which == "q":
                        nc.vector.tensor_scalar_add(
                            tl["qT"][:, t4 * TILE_N : (t4 + 1) * TILE_N],
                            acc[:], bq_sb[:])
                    elif which == "k":
                        nc.vector.tensor_scalar_add(
                            tl["kT"][:, t4 * TILE_N : (t4 + 1) * TILE_N],
                            acc[:], bk_sb[:])
                    else:
                        vt_sb = vstage.tile([128, TILE_N], MM_DT, tag="vt")
                        nc.vector.tensor_copy(vt_sb[:], acc[:])
                        # transpose [dims, tokens] -> [tokens, dims] on the
                        # PE; all four 128-blocks share one PSUM slot, then
                        # one fast high-priority copy releases it
                        ptr4 = ps_aux.tile([128, TILE_N], MM_DT, tag="aux")
                        for tt in range(TILE_N // 128):
                            nc.tensor.transpose(
                                ptr4[:, tt * 128 : (tt + 1) * 128],
                                vt_sb[:, tt * 128 : (tt + 1) * 128], ident[:])
                            yield
                        vUT = vstage.tile([128, TILE_N // 128, 128], MM_DT,
                                          tag="vUT", bufs=2)
                        nc.vector.tensor_copy(
                            vUT[:].rearrange("p b d -> p (b d)"), ptr4[:])
                        # v1 fp8 hi/lo split, whole 512-token chunk at once.
                        # Host pre-permuted V dims to [63, 0..62]; lo covers
                        # perm dims 1..63 and is written as fp8 directly.
                        vh = tl["v1"]
                        jsl4 = slice(t4 * 4, t4 * 4 + 4)
                        v4 = vUT[:].rearrange("p b (h d) -> p b h d", h=HPC)
                        # hi: fp8 round of v (all 64 perm dims)
                        nc.vector.tensor_copy(vh[:, jsl4, :, 0:D], v4)
                        # lo: v - hi (perm dims 1..63), fp8 output directly
                        nc.vector.tensor_sub(
                            vh[:, jsl4, :, D + 1 : 128],
                            v4[:, :, :, 1:D], vh[:, jsl4, :, 1:D])

        def gen_q_rest(b):
            """Generator: deferred q projections for chunks 1..3 of batch b."""
            tl = tiles[b]
            for t4 in range(1, NT4):
                xts = xhold.pop((b, t4))
                acc = ps_aux.tile([128, TILE_N], FP32, tag="aux")
                for kk in range(NK_C):
                    nc.tensor.matmul(
                        acc[:], wq_sb[:, kk * F : (kk + 1) * F], xts[kk][:],
                        start=(kk == 0), stop=(kk == NK_C - 1),
                    )
                    yield
                nc.vector.tensor_scalar_add(
                    tl["qT"][:, t4 * TILE_N : (t4 + 1) * TILE_N],
                    acc[:], bq_sb[:])

        def gen_wo_t4(b, t4):
            """Generator: output projection chunk; yields per PE op.
            PSUM drain runs on ACT (between exp tiles)."""
            t0 = b * T
            ctxT = tiles[b]["ctxT"]
            for o in range(C // 128):
                po = ps_aux.tile([128, TILE_N], FP32, tag="aux")
                nc.tensor.matmul(
                    po[:], wo_sb[:, o * 128 : (o + 1) * 128],
                    ctxT[:, t4 * TILE_N : (t4 + 1) * TILE_N],
                    start=True, stop=True,
                )
                osb = vstage.tile([128, TILE_N], OUT_DT, tag="osb", bufs=4)
                if OSB_PAT[o] == 'A':
                    nc.scalar.activation(osb[:], po[:], Copy)
                else:
                    nc.vector.tensor_copy(osb[:], po[:])
                nc.sync.dma_start(
                    outT[o * 128 : (o + 1) * 128,
                         t0 + t4 * TILE_N : t0 + (t4 + 1) * TILE_N],
                    osb[:],
                )
                yield

        fillers = []
        wo_hold = []

        def pull(budget):
            while budget > 0 and fillers:
                try:
                    next(fillers[0])
                    budget -= 1
                except StopIteration:
                    fillers.pop(0)

        # prologue: projections for batch 0 (PE-only ramp); batch 1's
        # x-loads and projection generator queue up behind it
        prefetch_x(0)
        prefetch_x(1)
        for _ in gen_proj(0, defer_q=True):
            pass
        fillers.append(gen_q_rest(0))
        fillers.append(gen_proj(1))

        # ---- flattened software-pipelined attention stream ----
        # one pair-step = 2 key tiles x 2 heads: 2x(QK pair + exp), then the
        # (a few steps earlier, per pv_sched) DoubleRow PV pair.  Chunks of
        # 512 queries flow back-to-back so no engine stalls at chunk borders.
        chunks = [(b, i4) for b in range(B) for i4 in range(NT4)]
        S = len(chunks) * NP
        pvs = {}     # chunk idx -> (pv0, pv1)
        expts = {}   # (chunk, pair) -> epair tile

        # pv pair -> emission step: finish each chunk's PV one step into the
        # next chunk so the normalize chain gets ~2 steps before the PSUM
        # banks are rewritten; chunk 0 lags more (v1 is still being built).
        from collections import defaultdict
        pv_sched = defaultdict(list)
        for c in range(len(chunks)):
            offs = [4, 5, 6, 7, 8, 9, 9, 10]
            for p, off in enumerate(offs):
                pv_sched[8 * c + off].append((c, p))
        S_END = max(pv_sched) + 1

        def chunk_start(ci):
            b, i4 = chunks[ci]
            if i4 == 0:
                tiles[b]["ctxT"] = bigs.tile([F, T], MM_DT, tag="ctxT",
                                             name=f"ctxT{b}", bufs=4)
            if b == B - 1 and i4 == 0:
                fillers.extend(wo_hold)
                wo_hold.clear()
            # queue the next batch's x loads (spread across pulls) and
            # projections at the start of this batch's attention;
            # batch 1 was queued by the prologue
            if i4 == 0 and 1 < b + 1 < B:
                fillers.append(gen_xt(b + 1))
                fillers.append(gen_proj(b + 1))

        def pv_step(ci, pp):
            b, i4 = chunks[ci]
            v1 = tiles[b]["v1"]
            if pp == 0:
                pvs[ci] = (ps_pv.tile([128, TILE_N], FP32, tag="pv0",
                                      name=f"pv0_{ci}"),
                           ps_pv.tile([128, TILE_N], FP32, tag="pv1",
                                      name=f"pv1_{ci}"))
            e = expts.pop((ci, pp))
            for h, pv in ((0, pvs[ci][0]), (1, pvs[ci][1])):
                nc.tensor.matmul(
                    pv[:], v1[:, 2 * pp : 2 * pp + 2, h, :],
                    e[:, :, h, :], start=(pp == 0), stop=(pp == NP - 1),
                    perf_mode=DR)
            if pp == NP - 1:
                chunk_finish(ci)

        def chunk_finish(ci):
            b, i4 = chunks[ci]
            isl = slice(i4 * TILE_N, (i4 + 1) * TILE_N)
            ctxT = tiles[b]["ctxT"]
            pv0, pv1 = pvs.pop(ci)
            # psum rows: 0:64 ctx_hi (perm dims), 64 denom, 65:128 ctx_lo
            # (perm dims 1..63).  hi+lo in one aligned stt per head; perm
            # dim 0 absorbs the denominator (+1, host-folded).  Bank is
            # released after its stt + recip reads.
            rd = small.tile([1, HPC, TILE_N], FP32, tag="rd")
            hs = small.tile([D, HPC, TILE_N], FP32, tag="hs")
            lo_s = small.tile([D, HPC, TILE_N], FP32, tag="lo_s")
            for h, pv in ((0, pv0), (1, pv1)):
                # partition-shift copy (quadrant base 64 -> 0), then the
                # hi+lo fold reads PSUM + SBUF at matching base 0
                nc.vector.tensor_copy(lo_s[:, h, :], pv[D:128, :])
                nc.vector.scalar_tensor_tensor(
                    hs[:, h, :], pv[0:D, :], 1.0, lo_s[:, h, :], MULT, ADD)
                # denom is row 0 of the shifted lo_s copy -> recip reads SBUF
                nc.vector.reciprocal_approx_fast(rd[:, h, :], lo_s[0:1, h, :])
            bc = small.tile([D, HPC, TILE_N], FP32, tag="bc")
            nc.gpsimd.partition_broadcast(bc[:], rd[:])
            for h in range(HPC):
                nc.vector.tensor_mul(
                    ctxT[h * D : (h + 1) * D, isl], hs[:, h, :], bc[:, h, :])
            if b == B - 1:
                if fillers:
                    fillers.insert(1, gen_wo_t4(b, i4))
                else:
                    fillers.append(gen_wo_t4(b, i4))
            elif i4 % 2 == 1:
                # defer half the wo work into the filler-starved last batch
                wo_hold.append(gen_wo_t4(b, i4))
            else:
                fillers.append(gen_wo_t4(b, i4))

        for s in range(S_END):
            due = pv_sched.get(s, [])
            if s < S:
                ci, p = divmod(s, NP)
                b, i4 = chunks[ci]
                if p == 0:
                    chunk_start(ci)
                qT, kT = tiles[b]["qT"], tiles[b]["kT"]
                isl = slice(i4 * TILE_N, (i4 + 1) * TILE_N)
                epair = epool.tile([128, 2, HPC, TILE_N], PV_DT, tag="expt")
                expts[ci, p] = epair
                for j2 in range(2):
                    j = 2 * p + j2
                    jsl = slice(j * TILE_K, (j + 1) * TILE_K)
                    qk = ps_qk.tile([128, HPC, TILE_N], FP32, tag="qk")
                    # heads in distinct PE row-groups -> run concurrently
                    nc.tensor.matmul(qk[:, 0, :], kT[0:D, jsl],
                                     qT[0:D, isl], start=True, stop=True)
                    nc.tensor.matmul(qk[:, 1, :],
                                     kT[D : 2 * D, jsl], qT[D : 2 * D, isl],
                                     start=True, stop=True)
                    eng = EXP_PAT[(p * 2 + j2) % 16]
                    eslice = epair[:, j2]
                    if eng == 'A':
                        nc.scalar.activation(eslice, qk[:], Exp,
                                             bias=nexp[:], scale=EXP_SCALE)
                    else:
                        nc.vector.tensor_scalar(
                            eslice.bitcast(I8), qk[:], B_DVE, 0.0, ADD, MAX)
                    # keep the in-order PE stream fed while exp runs
                    if j2 == 0:
                        for c2, p2 in due[0:1]:
                            pv_step(c2, p2)
                    else:
                        for c2, p2 in due[1:]:
                            pv_step(c2, p2)
                    pull(2 if j2 == 0 else 3)
            else:
                for c2, p2 in due:
                    pv_step(c2, p2)
                pull(6)

        # drain remaining fillers (last batch's final wo chunks)
        fillers.extend(wo_hold)
        wo_hold.clear()
        pull(10 ** 9)


_CACHE = {}


def _get_nc():
    if "nc" not in _CACHE:
        nc = bacc.Bacc("TRN2", target_bir_lowering=False, debug=False,
                       num_devices=NCORES)
        with tile.TileContext(nc) as tc:
            build_kernel_body(tc)
        nc.compile()
        _CACHE["nc"] = nc
    return _CACHE["nc"]


# per-head dim permutation [63, 0..62]: perm-dim 0 absorbs the denominator
_PERM = np.concatenate([[D - 1], np.arange(D - 1)])


def host_prep(x, Wq, bq, Wk, bk, Wv, bv, Wo, bo):
    f16 = np.float16
    x = np.asarray(x, np.float32)
    xT = np.ascontiguousarray(x.reshape(TT, C).T.astype(f16))
    scale = np.float32(A_BITS / np.sqrt(D))
    Wv32 = np.asarray(Wv, np.float32)
    Wo32 = np.asarray(Wo, np.float32)
    # permute V output dims (and matching Wo input dims) per head
    perm_all = np.concatenate([h * D + _PERM for h in range(H)])
    Wv_p = Wv32[:, perm_all]
    Wo_p = Wo32[perm_all, :]
    in_maps = []
    for c in range(NCORES):
        fsl = slice(c * F, (c + 1) * F)
        in_maps.append({
            "xT": xT,
            "wq": np.ascontiguousarray(
                (np.asarray(Wq, np.float32)[:, fsl] * scale).astype(f16)),
            "wk": np.ascontiguousarray(np.asarray(Wk, np.float32)[:, fsl].astype(f16)),
            "wv": np.ascontiguousarray(Wv_p[:, fsl].astype(f16)),
            "wo": np.ascontiguousarray(Wo_p[fsl, :].astype(f16)),
            "bq": np.ascontiguousarray(np.asarray(bq, np.float32)[fsl] * scale),
            "bk": np.ascontiguousarray(np.asarray(bk, np.float32)[fsl]),
        })
    return in_maps


def host_gather(results, Wo, bo, bv):
    total = np.zeros((C, TT), np.float32)
    for c in range(NCORES):
        total += results[c]["outT"].astype(np.float32)
    out = total.T
    Wo32 = np.asarray(Wo, np.float32)
    # +1 artifact on perm-dim 0 (orig d63) of every head
    ones_fix = Wo32[[h * D + (D - 1) for h in range(H)], :].sum(axis=0)
    out = out + (np.asarray(bo, np.float32)
                 + np.asarray(bv, np.float32) @ Wo32 - ones_fix)
    return out.reshape(B, T, C)


def _install_profile_hook():
    """Make trace=True work under axon when antenv.axon_hooks is absent."""
    import sys
    import types

    try:
        import antenv.axon_hooks  # noqa: F401
        return
    except ImportError:
        pass
    import antenv
    from trn_agent_boot.trn_boot import _ntff_profile_via_ctypes

    mod = types.ModuleType("antenv.axon_hooks")
    holder = [None]
    mod.set_axon_ntff_profile_hook = lambda h: holder.__setitem__(0, h)
    mod.get_axon_ntff_profile_hook = lambda: holder[0]
    sys.modules["antenv.axon_hooks"] = mod
    antenv.axon_hooks = mod
    mod.set_axon_ntff_profile_hook(
        _ntff_profile_via_ctypes("/opt/axon/libaxon_pjrt.so")
    )
    # artifact upload needs internal storage; keep profiles local
    import concourse.bass_utils as bu
    bu.upload_artifacts = lambda tmpdir: f"local:{tmpdir}"


def kernel(x, Wq, bq, Wk, bk, Wv, bv, Wo, bo, _trace=False):
    if _trace:
        _install_profile_hook()
    nc = _get_nc()
    in_maps = host_prep(x, Wq, bq, Wk, bk, Wv, bv, Wo, bo)
    res = run_bass_kernel_spmd(nc, in_maps, core_ids=list(range(NCORES)),
                               trace=_trace)
    _CACHE["last_result"] = res
    return host_gather(res.results, Wo, bo, bv)


# revision 22
# speedup vs baseline: 1.2090x; 1.0303x over previous
"""Multi-head self-attention (B=4, T=2048, C=1024, H=16, D=64) on 8 NeuronCores.

Sharding: tensor-parallel over heads (Megatron): each core owns 2 heads.
Wq/Wk/Wv column-sharded, Wo row-sharded; host sums the 8 partial outputs.

Device layout is fully "transposed" (features on partitions, tokens on the
free dim) so that softmax runs over the PSUM free dim and the PV matmul needs
no attention-matrix transpose.

Softmax exp is computed three ways, split across engines to break the
Scalar-engine bottleneck:
  - ACT: exact exp -> fp8e4 (activation with scale=1/A, bias shift)
  - DVE/Pool: Schraudolph fast-exp -> int8 bitcast fp8e4:
      bits = clamp(round(A*score + B), 0, 127);  A = 8*log2(e) folded into
      Wq on the host, so the op is a single tensor_scalar add+max with int8
      output.  The int8 bit pattern IS the fp8e4m3 encoding of
      ~exp(score)*2^((B-56)/8), on the same quantization grid the ACT path
      produces; the only extra error is the secant interpolation (~2% rms).
The per-query denominator comes from a stationary 'ones' column in the PV
matmul, so ACT/DVE/Pool tiles stay mutually consistent.

PV runs fp8e4 DoubleRow (2 key-tiles per matmul).  V is split hi/lo with the
128 stationary columns [v_hi(64,perm) | ones(1) | v_lo(63)] where the host
pre-permutes V/Wo dims per head to [63, 0..62].  This makes the PSUM row map
  rows 0:64  = ctx_hi (perm dims 0..63)
  row  64    = denominator
  rows 65:128= ctx_lo (perm dims 1..63)
so hi+lo correction is ONE aligned scalar_tensor_tensor:
  hs = pv[0:64] + pv[64:128]
Perm-dim 0 (orig d63) instead absorbs the denominator: ctx'_0 = ctx_0 + 1,
a constant that the host folds into the output bias (minus sum of the
corresponding Wo rows).

The PE executes its stream in order, so projection/output-projection matmuls
of adjacent batches are interleaved into the attention j-loop to keep the PE
near 100% busy; Wo PSUM drains run on ACT between exp tiles.
"""

import numpy as np

import concourse.bass as bass
import concourse.tile as tile
from concourse import bacc, mybir
from concourse.bass_utils import run_bass_kernel_spmd

B, T, C, H, D = 4, 2048, 1024, 16, 64
NCORES = 8
HPC = H // NCORES          # heads per core = 2
F = HPC * D                # per-core feature width = 128
TT = B * T                 # total tokens = 8192

FP32 = mybir.dt.float32
FP16 = mybir.dt.float16
I8 = mybir.dt.int8
MM_DT = mybir.dt.float16   # matmul compute dtype
PV_DT = mybir.dt.float8e4  # PV matmul dtype (DoubleRow: 2 key-tiles per MM)
OUT_DT = mybir.dt.float16  # partial-output DMA dtype

TILE_K = 128               # contraction tile
TILE_N = 512               # moving free dim per matmul
NK_C = C // TILE_K         # 8 k-tiles over channels
NT4 = T // TILE_N          # 4 token chunks per batch
NJ = T // TILE_K           # 16 key tiles per batch
NP = NJ // 2               # 8 key-tile PAIRS per batch (DoubleRow)
NLO = D - 1                # 63 lo-correction dims (perm-dim 0 is hi-only)

A_BITS = 8.0 / np.log(2.0)          # 11.5416: fp8e4m3 bits per ln unit
B_BITS = 24.0                       # bias anchor: max P byte 118 < 0x78
# CRITICAL: the PE fp8e4m3 decoder treats exponent-field 1111 (any byte
# >= 0x78, value >= 256) as Inf/NaN -- P values must stay < 256.
# DVE fast-exp bias, recentered by the mean secant error so DVE tiles match
# the ACT-exact tiles' scale inside one softmax row (swept in simulation).
B_DVE = 23.6
EXP_BIAS = (B_BITS - 56.0) / 8.0 * np.log(2.0)  # ACT-exp shift, same scale
EXP_SCALE = 1.0 / A_BITS            # undo the A folded into Wq

# exp-engine rotation per chunk: 16 tiles (8 pair-steps x 2 j2)
# A=ACT exact exp, D=DVE fast-exp (Pool cannot read PSUM)
EXP_PAT = ['A', 'D', 'A', 'A', 'D', 'A', 'A', 'D',
           'A', 'A', 'D', 'A', 'A', 'D', 'A', 'A']
# Wo PSUM drain engine per o-tile (6 ACT / 2 DVE per chunk)
OSB_PAT = ['A', 'D', 'A', 'D', 'A', 'D', 'A', 'D']
# PV consumption lag per pair within a chunk (steps after the QK step)
PV_OFFS = [5, 6, 7, 8, 9, 10, 10, 11]
# filler-pull budgets (after j2=0 exp, after j2=1 exp, in the drain region)
PULLS = (2, 3, 6)
EPOOL_BUFS = 10
MULT_ENG = 'D'          # ctx normalize multiply: 'D'=DVE, 'P'=Pool
WO_DEFER = 2            # defer wo chunks with i4 % WO_DEFER == 1 to last batch


def build_kernel_body(tc):
    nc = tc.nc
    Exp = mybir.ActivationFunctionType.Exp
    Copy = mybir.ActivationFunctionType.Copy
    Ident = mybir.ActivationFunctionType.Identity
    DR = mybir.MatmulPerfMode.DoubleRow
    ADD = mybir.AluOpType.add
    MAX = mybir.AluOpType.max
    MULT = mybir.AluOpType.mult

    xT = nc.dram_tensor("xT", [C, TT], MM_DT, kind="ExternalInput").ap()
    wq = nc.dram_tensor("wq", [C, F], MM_DT, kind="ExternalInput").ap()
    wk = nc.dram_tensor("wk", [C, F], MM_DT, kind="ExternalInput").ap()
    wv = nc.dram_tensor("wv", [C, F], MM_DT, kind="ExternalInput").ap()
    wo = nc.dram_tensor("wo", [F, C], MM_DT, kind="ExternalInput").ap()
    bqv = nc.dram_tensor("bq", [F], FP32, kind="ExternalInput").ap()
    bkv = nc.dram_tensor("bk", [F], FP32, kind="ExternalInput").ap()
    outT = nc.dram_tensor("outT", [C, TT], OUT_DT, kind="ExternalOutput").ap()


    import contextlib
    ctx = contextlib.ExitStack()
    with ctx:
        consts = ctx.enter_context(tc.tile_pool(name="consts", bufs=1))
        xpool = ctx.enter_context(tc.tile_pool(name="xt", bufs=64))
        bigs = ctx.enter_context(tc.tile_pool(name="bigs", bufs=2))
        epool = ctx.enter_context(tc.tile_pool(name="expp", bufs=EPOOL_BUFS))
        small = ctx.enter_context(tc.tile_pool(name="small", bufs=2))
        vstage = ctx.enter_context(tc.tile_pool(name="vstage", bufs=2))
        ps_qk = ctx.enter_context(tc.tile_pool(name="ps_qk", bufs=2, space="PSUM"))
        ps_pv = ctx.enter_context(tc.tile_pool(name="ps_pv", bufs=1, space="PSUM"))
        ps_aux = ctx.enter_context(tc.tile_pool(name="ps_aux", bufs=2, space="PSUM"))

        # ---- constants ----
        wq_sb = consts.tile([TILE_K, C], MM_DT)  # c-tile k at [:, k*F:(k+1)*F]
        nc.sync.dma_start(
            wq_sb[:].rearrange("p (k f) -> p k f", k=NK_C),
            wq.rearrange("(k p) f -> p k f", p=TILE_K))
        wk_sb = consts.tile([TILE_K, C], MM_DT)
        nc.sync.dma_start(
            wk_sb[:].rearrange("p (k f) -> p k f", k=NK_C),
            wk.rearrange("(k p) f -> p k f", p=TILE_K))
        wv_sb = consts.tile([TILE_K, C], MM_DT)
        nc.sync.dma_start(
            wv_sb[:].rearrange("p (k f) -> p k f", p=TILE_K, k=NK_C),
            wv.rearrange("(k p) f -> p k f", p=TILE_K))
        wo_sb = consts.tile([F, C], MM_DT)
        nc.sync.dma_start(wo_sb[:], wo)
        bq_sb = consts.tile([F, 1], FP32)
        nc.sync.dma_start(bq_sb[:], bqv.rearrange("(p one) -> p one", one=1))
        bk_sb = consts.tile([F, 1], FP32)
        nc.sync.dma_start(bk_sb[:], bkv.rearrange("(p one) -> p one", one=1))
        ident32 = consts.tile([128, 128], FP32)
        from concourse.masks import make_identity
        make_identity(nc, ident32[:])
        ident = consts.tile([128, 128], MM_DT)
        nc.vector.tensor_copy(ident[:], ident32[:])
        ones8 = consts.tile([128, NJ * HPC], PV_DT)
        nc.gpsimd.memset(ones8[:], 1.0)
        nexp = consts.tile([128, 1], FP32)
        nc.gpsimd.memset(nexp[:], float(EXP_BIAS))

        tiles = {}  # per-batch SBUF tiles

        def alloc_proj_tiles(b):
            qT = bigs.tile([F, T], MM_DT, tag="qT", name=f"qT{b}")
            kT = bigs.tile([F, T], MM_DT, tag="kT", name=f"kT{b}")
            # PV stationary, both heads: [keys, ktile, head, hi|ones|lo]
            v1 = bigs.tile([128, NJ, HPC, 128], PV_DT, tag="v1",
                           name=f"v1_{b}")
            nc.vector.tensor_copy(
                v1[:, :, :, D : D + 1],
                ones8[:].rearrange("p (a h b) -> p a h b", h=HPC, b=1),
            )
            tiles[b] = {"qT": qT, "kT": kT, "v1": v1}

        xstage = {}

        def gen_xt(b):
            """Generator: issue batch b's x DMA loads, spread out so the
            Sync queue is not clogged by a 32-DMA burst."""
            t0 = b * T
            for t4 in range(NT4):
                for kk in range(NK_C):
                    xt = xpool.tile([TILE_K, TILE_N], MM_DT, tag="xt",
                                    name=f"xt{b}_{t4}_{kk}")
                    nc.sync.dma_start(
                        xt[:],
                        xT[kk * TILE_K : (kk + 1) * TILE_K,
                           t0 + t4 * TILE_N : t0 + (t4 + 1) * TILE_N],
                    )
                    xstage[b, t4, kk] = xt
                    if kk % 4 == 3:
                        yield

        def prefetch_x(b):
            for _ in gen_xt(b):
                pass

        xhold = {}

        def gen_proj(b, defer_q=False):
            """Generator: projections for batch b; yields after each PE op."""
            alloc_proj_tiles(b)
            tl = tiles[b]
            for t4 in range(NT4):
                xts = [xstage.pop((b, t4, kk))[:] for kk in range(NK_C)]
                wlist = (("q", wq_sb), ("k", wk_sb), ("v", wv_sb))
                if defer_q and t4 > 0:
                    xhold[b, t4] = xts
                    wlist = (("k", wk_sb), ("v", wv_sb))
                for which, w_sb in wlist:
                    acc = ps_aux.tile([128, TILE_N], FP32, tag="aux")
                    for kk in range(NK_C):
                        nc.tensor.matmul(
                            acc[:], w_sb[:, kk * F : (kk + 1) * F], xts[kk],
                            start=(kk == 0), stop=(kk == NK_C - 1),
                        )
                        yield
                    if which == "q":
                        nc.vector.tensor_scalar_add(
                            tl["qT"][:, t4 * TILE_N : (t4 + 1) * TILE_N],
                            acc[:], bq_sb[:])
                    elif which == "k":
                        nc.vector.tensor_scalar_add(
                            tl["kT"][:, t4 * TILE_N : (t4 + 1) * TILE_N],
                            acc[:], bk_sb[:])
                    else:
                        vt_sb = vstage.tile([128, TILE_N], MM_DT, tag="vt")
                        nc.vector.tensor_copy(vt_sb[:], acc[:])
                        # transpose [dims, tokens] -> [tokens, dims] on the
                        # PE; all four 128-blocks share one PSUM slot, then
                        # one fast high-priority copy releases it
                        ptr4 = ps_aux.tile([128, TILE_N], MM_DT, tag="aux")
                        for tt in range(TILE_N // 128):
                            nc.tensor.transpose(
                                ptr4[:, tt * 128 : (tt + 1) * 128],
                                vt_sb[:, tt * 128 : (tt + 1) * 128], ident[:])
                            yield
                        vUT = vstage.tile([128, TILE_N // 128, 128], MM_DT,
                                          tag="vUT", bufs=2)
                        nc.vector.tensor_copy(
                            vUT[:].rearrange("p b d -> p (b d)"), ptr4[:])
                        # v1 fp8 hi/lo split, whole 512-token chunk at once.
                        # Host pre-permuted V dims to [63, 0..62]; lo covers
                        # perm dims 1..63 and is written as fp8 directly.
                        vh = tl["v1"]
                        jsl4 = slice(t4 * 4, t4 * 4 + 4)
                        v4 = vUT[:].rearrange("p b (h d) -> p b h d", h=HPC)
                        # hi: fp8 round of v (all 64 perm dims)
                        nc.vector.tensor_copy(vh[:, jsl4, :, 0:D], v4)
                        # lo: v - hi (perm dims 1..63), fp8 output directly
                        nc.vector.tensor_sub(
                            vh[:, jsl4, :, D + 1 : 128],
                            v4[:, :, :, 1:D], vh[:, jsl4, :, 1:D])

        def gen_q_rest(b):
            """Generator: deferred q projections for chunks 1..3 of batch b."""
            tl = tiles[b]
            for t4 in range(1, NT4):
                xts = xhold.pop((b, t4))
                acc = ps_aux.tile([128, TILE_N], FP32, tag="aux")
                for kk in range(NK_C):
                    nc.tensor.matmul(
                        acc[:], wq_sb[:, kk * F : (kk + 1) * F], xts[kk],
                        start=(kk == 0), stop=(kk == NK_C - 1),
                    )
                    yield
                nc.vector.tensor_scalar_add(
                    tl["qT"][:, t4 * TILE_N : (t4 + 1) * TILE_N],
                    acc[:], bq_sb[:])

        def gen_wo_t4(b, t4):
            """Generator: output projection chunk; yields per PE op.
            PSUM drain runs on ACT (between exp tiles)."""
            t0 = b * T
            ctxT = tiles[b]["ctxT"]
            for o in range(C // 128):
                po = ps_aux.tile([128, TILE_N], FP32, tag="aux")
                nc.tensor.matmul(
                    po[:], wo_sb[:, o * 128 : (o + 1) * 128],
                    ctxT[:, t4 * TILE_N : (t4 + 1) * TILE_N],
                    start=True, stop=True,
                )
                osb = vstage.tile([128, TILE_N], OUT_DT, tag="osb", bufs=4)
                if OSB_PAT[o] == 'A':
                    nc.scalar.activation(osb[:], po[:], Copy)
                else:
                    nc.vector.tensor_copy(osb[:], po[:])
                nc.sync.dma_start(
                    outT[o * 128 : (o + 1) * 128,
                         t0 + t4 * TILE_N : t0 + (t4 + 1) * TILE_N],
                    osb[:],
                )
                yield

        fillers = []
        wo_hold = []

        def pull(budget):
            while budget > 0 and fillers:
                try:
                    next(fillers[0])
                    budget -= 1
                except StopIteration:
                    fillers.pop(0)

        # prologue: projections for batch 0 (PE-only ramp); batch 1's
        # x-loads and projection generator queue up behind it
        prefetch_x(0)
        prefetch_x(1)
        for _ in gen_proj(0, defer_q=True):
            pass
        fillers.append(gen_q_rest(0))
        fillers.append(gen_proj(1))

        # ---- flattened software-pipelined attention stream ----
        # one pair-step = 2 key tiles x 2 heads: 2x(QK pair + exp), then the
        # (a few steps earlier, per pv_sched) DoubleRow PV pair.  Chunks of
        # 512 queries flow back-to-back so no engine stalls at chunk borders.
        chunks = [(b, i4) for b in range(B) for i4 in range(NT4)]
        S = len(chunks) * NP
        pvs = {}     # chunk idx -> (pv0, pv1)
        expts = {}   # (chunk, pair) -> epair tile

        # pv pair -> emission step: finish each chunk's PV one step into the
        # next chunk so the normalize chain gets ~2 steps before the PSUM
        # banks are rewritten; chunk 0 lags more (v1 is still being built).
        from collections import defaultdict
        pv_sched = defaultdict(list)
        for c in range(len(chunks)):
            offs = PV_OFFS
            for p, off in enumerate(offs):
                pv_sched[8 * c + off].append((c, p))
        S_END = max(pv_sched) + 1

        def chunk_start(ci):
            b, i4 = chunks[ci]
            if i4 == 0:
                tiles[b]["ctxT"] = bigs.tile([F, T], MM_DT, tag="ctxT",
                                             name=f"ctxT{b}", bufs=4)
            if b == B - 1 and i4 == 0:
                fillers.extend(wo_hold)
                wo_hold.clear()
            # queue the next batch's x loads (spread across pulls) and
            # projections at the start of this batch's attention;
            # batch 1 was queued by the prologue
            if i4 == 0 and 1 < b + 1 < B:
                fillers.append(gen_xt(b + 1))
                fillers.append(gen_proj(b + 1))

        def pv_step(ci, pp):
            b, i4 = chunks[ci]
            v1 = tiles[b]["v1"]
            if pp == 0:
                pvs[ci] = (ps_pv.tile([128, TILE_N], FP32, tag="pv0",
                                      name=f"pv0_{ci}"),
                           ps_pv.tile([128, TILE_N], FP32, tag="pv1",
                                      name=f"pv1_{ci}"))
            e = expts.pop((ci, pp))
            for h, pv in ((0, pvs[ci][0]), (1, pvs[ci][1])):
                nc.tensor.matmul(
                    pv[:], v1[:, 2 * pp : 2 * pp + 2, h, :],
                    e[:, :, h, :], start=(pp == 0), stop=(pp == NP - 1),
                    perf_mode=DR)
            if pp == NP - 1:
                chunk_finish(ci)

        def chunk_finish(ci):
            b, i4 = chunks[ci]
            isl = slice(i4 * TILE_N, (i4 + 1) * TILE_N)
            ctxT = tiles[b]["ctxT"]
            pv0, pv1 = pvs.pop(ci)
            # psum rows: 0:64 ctx_hi (perm dims), 64 denom, 65:128 ctx_lo
            # (perm dims 1..63).  hi+lo in one aligned stt per head; perm
            # dim 0 absorbs the denominator (+1, host-folded).  Bank is
            # released after its stt + recip reads.
            rd = small.tile([1, HPC, TILE_N], FP32, tag="rd")
            hs = small.tile([D, HPC, TILE_N], FP32, tag="hs")
            lo_s = small.tile([D, HPC, TILE_N], FP32, tag="lo_s")
            for h, pv in ((0, pv0), (1, pv1)):
                # partition-shift copy (quadrant base 64 -> 0), then the
                # hi+lo fold reads PSUM + SBUF at matching base 0
                nc.vector.tensor_copy(lo_s[:, h, :], pv[D:128, :])
                nc.vector.scalar_tensor_tensor(
                    hs[:, h, :], pv[0:D, :], 1.0, lo_s[:, h, :], MULT, ADD)
                # denom is row 0 of the shifted lo_s copy -> recip reads SBUF
                nc.vector.reciprocal_approx_fast(rd[:, h, :], lo_s[0:1, h, :])
            bc = small.tile([D, HPC, TILE_N], FP32, tag="bc")
            nc.gpsimd.partition_broadcast(bc[:], rd[:])
            for h in range(HPC):
                if MULT_ENG == 'P':
                    nc.gpsimd.tensor_tensor(
                        ctxT[h * D : (h + 1) * D, isl], hs[:, h, :],
                        bc[:, h, :], MULT)
                else:
                    nc.vector.tensor_mul(
                        ctxT[h * D : (h + 1) * D, isl], hs[:, h, :],
                        bc[:, h, :])
            if b == B - 1:
                if fillers:
                    fillers.insert(1, gen_wo_t4(b, i4))
                else:
                    fillers.append(gen_wo_t4(b, i4))
            elif WO_DEFER and i4 % WO_DEFER == 1:
                # defer part of the wo work into the filler-starved last batch
                wo_hold.append(gen_wo_t4(b, i4))
            else:
                fillers.append(gen_wo_t4(b, i4))

        for s in range(S_END):
            due = pv_sched.get(s, [])
            if s < S:
                ci, p = divmod(s, NP)
                b, i4 = chunks[ci]
                if p == 0:
                    chunk_start(ci)
                qT, kT = tiles[b]["qT"], tiles[b]["kT"]
                isl = slice(i4 * TILE_N, (i4 + 1) * TILE_N)
                epair = epool.tile([128, 2, HPC, TILE_N], PV_DT, tag="expt")
                expts[ci, p] = epair
                for j2 in range(2):
                    j = 2 * p + j2
                    jsl = slice(j * TILE_K, (j + 1) * TILE_K)
                    qk = ps_qk.tile([128, HPC, TILE_N], FP32, tag="qk")
                    # heads in distinct PE row-groups -> run concurrently
                    nc.tensor.matmul(qk[:, 0, :], kT[0:D, jsl],
                                     qT[0:D, isl], start=True, stop=True)
                    nc.tensor.matmul(qk[:, 1, :],
                                     kT[D : 2 * D, jsl], qT[D : 2 * D, isl],
                                     start=True, stop=True)
                    eng = EXP_PAT[(p * 2 + j2) % 16]
                    eslice = epair[:, j2]
                    if eng == 'A':
                        nc.scalar.activation(eslice, qk[:], Exp,
                                             bias=nexp[:], scale=EXP_SCALE)
                    else:
                        nc.vector.tensor_scalar(
                            eslice.bitcast(I8), qk[:], B_DVE, 0.0, ADD, MAX)
                    # keep the in-order PE stream fed while exp runs
                    if j2 == 0:
                        for c2, p2 in due[0:1]:
                            pv_step(c2, p2)
                    else:
                        for c2, p2 in due[1:]:
                            pv_step(c2, p2)
                    pull(PULLS[0] if j2 == 0 else PULLS[1])
            else:
                for c2, p2 in due:
                    pv_step(c2, p2)
                pull(PULLS[2])

        # drain remaining fillers (last batch's final wo chunks)
        fillers.extend(wo_hold)
        wo_hold.clear()
        pull(10 ** 9)


_CACHE = {}


def _get_nc():
    if "nc" not in _CACHE:
        nc = bacc.Bacc("TRN2", target_bir_lowering=False, debug=False,
                       num_devices=NCORES)
        with tile.TileContext(nc) as tc:
            build_kernel_body(tc)
        nc.compile()
        _CACHE["nc"] = nc
    return _CACHE["nc"]


# per-head dim permutation [63, 0..62]: perm-dim 0 absorbs the denominator
_PERM = np.concatenate([[D - 1], np.arange(D - 1)])


def host_prep(x, Wq, bq, Wk, bk, Wv, bv, Wo, bo):
    f16 = np.float16
    x = np.asarray(x, np.float32)
    xT = np.ascontiguousarray(x.reshape(TT, C).T.astype(f16))
    scale = np.float32(A_BITS / np.sqrt(D))
    Wv32 = np.asarray(Wv, np.float32)
    Wo32 = np.asarray(Wo, np.float32)
    # permute V output dims (and matching Wo input dims) per head
    perm_all = np.concatenate([h * D + _PERM for h in range(H)])
    Wv_p = Wv32[:, perm_all]
    Wo_p = Wo32[perm_all, :]
    in_maps = []
    for c in range(NCORES):
        fsl = slice(c * F, (c + 1) * F)
        in_maps.append({
            "xT": xT,
            "wq": np.ascontiguousarray(
                (np.asarray(Wq, np.float32)[:, fsl] * scale).astype(f16)),
            "wk": np.ascontiguousarray(np.asarray(Wk, np.float32)[:, fsl].astype(f16)),
            "wv": np.ascontiguousarray(Wv_p[:, fsl].astype(f16)),
            "wo": np.ascontiguousarray(Wo_p[fsl, :].astype(f16)),
            "bq": np.ascontiguousarray(np.asarray(bq, np.float32)[fsl] * scale),
            "bk": np.ascontiguousarray(np.asarray(bk, np.float32)[fsl]),
        })
    return in_maps


def host_gather(results, Wo, bo, bv):
    total = np.zeros((C, TT), np.float32)
    for c in range(NCORES):
        total += results[c]["outT"].astype(np.float32)
    out = total.T
    Wo32 = np.asarray(Wo, np.float32)
    # +1 artifact on perm-dim 0 (orig d63) of every head
    ones_fix = Wo32[[h * D + (D - 1) for h in range(H)], :].sum(axis=0)
    out = out + (np.asarray(bo, np.float32)
                 + np.asarray(bv, np.float32) @ Wo32 - ones_fix)
    return out.reshape(B, T, C)


def _install_profile_hook():
    """Make trace=True work under axon when antenv.axon_hooks is absent."""
    import sys
    import types

    try:
        import antenv.axon_hooks  # noqa: F401
        return
    except ImportError:
        pass
    import antenv
    from trn_agent_boot.trn_boot import _ntff_profile_via_ctypes

    mod = types.ModuleType("antenv.axon_hooks")
    holder = [None]
    mod.set_axon_ntff_profile_hook = lambda h: holder.__setitem__(0, h)
    mod.get_axon_ntff_profile_hook = lambda: holder[0]
    sys.modules["antenv.axon_hooks"] = mod
    antenv.axon_hooks = mod
    mod.set_axon_ntff_profile_hook(
        _ntff_profile_via_ctypes("/opt/axon/libaxon_pjrt.so")
    )
    # artifact upload needs internal storage; keep profiles local
    import concourse.bass_utils as bu
    bu.upload_artifacts = lambda tmpdir: f"local:{tmpdir}"


def kernel(x, Wq, bq, Wk, bk, Wv, bv, Wo, bo, _trace=False):
    if _trace:
        _install_profile_hook()
    nc = _get_nc()
    in_maps = host_prep(x, Wq, bq, Wk, bk, Wv, bv, Wo, bo)
    res = run_bass_kernel_spmd(nc, in_maps, core_ids=list(range(NCORES)),
                               trace=_trace)
    _CACHE["last_result"] = res
    return host_gather(res.results, Wo, bo, bv)
